# revision 19
# baseline (speedup 1.0000x reference)
"""Trainium2 Bass kernel: 3x3 stride-1 pad-1 conv2d, N=16,Cin=64,Cout=128,H=W=224.

Sharding: data-parallel over batch: 8 cores x 2 images each.

Per-core algorithm (v2 — fully resident pre-padded input):
  - Input is pre-padded ON HOST to [2, 64, 226, 225] fp16: row 0 / row 225
    are the zero top/bottom halo, col 224 of every row is the zero pad
    column. The whole image pair lives in one SBUF tile [128, 1+226*225]
    (partitions 0-63 = img0 channels, 64-127 = img1), so every band load
    is a single fully-contiguous per-partition DMA (~16KB packets instead
    of the 450B per-row packets a strided SBUF destination forces).
    One guard element at tile index 0 covers the (y=0,dr=0,dc=0) tap.
  - With the flat 225-stride layout, tap (dr,dc) for output rows [y,y+1]
    is the contiguous stream starting at (y+dr)*225 + dc: the pad column
    doubles as the left/right border zero, so no fix-up passes.
  - conv = sum over 9 taps of fp16 matmuls into PSUM:
      psum[co, 448] += w[ci, tap, co].T @ xt[ci, base..base+450 as 2x225, :224]
    K=64 (Cin), M=128 (Cout), N=448 (2 output rows, one PSUM bank).
    img0 matmuls use PE rows 0-63, img1 rows 64-127 (tile_position from
    base partitions) -> the two streams run concurrently in disjoint
    row-groups of the systolic array.
  - PSUM chunk [128, 448] evicted to bf16 SBUF staging with fused bias
    add (img0 on DVE, img1 on ACT); bf16 output halves store traffic.
    Stores go out in 8-row pieces on the otherwise-idle gpsimd queue — a
    waiting store kick must never head-of-line-block ACT evictions or
    input loads, since matmuls wait on evictions to recycle PSUM banks.
  - A few warm-up matmuls on zero scratch right after the fixed ~6us
    preamble start the PE DVFS clock ramp before real data arrives.
"""

import numpy as np

N_IMG, C_IN, C_OUT, KS, H, W = 16, 64, 128, 3, 224, 224
N_CORES = 8
IMGS_PER_CORE = N_IMG // N_CORES  # 2
WP = W + 1  # padded row stride (225)
HP = H + 2  # padded rows incl. top/bottom halo (226)
TAPS = [(dr, dc) for dr in range(KS) for dc in range(KS)]


def build_conv_program(evict_split=3, out_bf16=True):
    import concourse.bacc as bacc
    import concourse.mybir as mybir
    import concourse.tile as tile

    h, w, wp = H, W, WP
    r = 32  # output rows per staging band
    n_bands = h // r
    n_chunk = r // 2
    chunk = 2 * w  # 448
    flat = r * w
    f32 = mybir.dt.float32
    f16 = mybir.dt.float16
    odt = mybir.dt.bfloat16 if out_bf16 else f32
    # guard + padded image + 2 slack elems so the last tap's 450-wide
    # slice (base=224*225+2) stays in bounds (its view reads only :224).
    xt_len = 1 + HP * wp + 2

    nc = bacc.Bacc("TRN2", target_bir_lowering=False)

    x_d = nc.dram_tensor("x", [IMGS_PER_CORE, C_IN, HP, wp], f16, kind="ExternalInput")
    w_d = nc.dram_tensor("w", [C_IN, 9, C_OUT], f16, kind="ExternalInput")
    b_d = nc.dram_tensor("bias", [C_OUT, 1], f32, kind="ExternalInput")
    out_d = nc.dram_tensor(
        "out", [IMGS_PER_CORE, C_OUT, h, w], odt, kind="ExternalOutput"
    )

    with tile.TileContext(nc) as tc:
        with (
            tc.tile_pool(name="const", bufs=1) as const_pool,
            tc.tile_pool(name="outs", bufs=2) as o_pool,
            tc.tile_pool(name="psum", bufs=8, space="PSUM") as p_pool,
        ):
            # fp16 weights: per-matmul LDWEIGHTS hides under the N=448
            # moving stream. PE upconverts to fp22; accumulation is fp32.
            w_sb = const_pool.tile([128, 9, C_OUT], f16)
            nc.sync.dma_start(out=w_sb[0:64], in_=w_d[:])
            nc.scalar.dma_start(out=w_sb[64:128], in_=w_d[:])
            bias_sb = const_pool.tile([C_OUT, 1], f32)
            nc.sync.dma_start(out=bias_sb[:], in_=b_d[:])

            xt = const_pool.tile([128, xt_len], f16)
            nc.vector.memset(xt[:, 0:1], 0.0)  # guard elem (y=0,dr=0,dc=0 tap)

            # Warm-up matmuls on a zero scratch tile: the PE DVFS ramp
            # takes tens of us to reach full clock, so start it during the
            # input-load window instead of paying it on the real stream.
            scratch = const_pool.tile([128, chunk], f16)
            nc.vector.memset(scratch[:], 0.0)
            warm_ps = p_pool.tile([C_OUT, chunk], f32, tag="ps", bufs=8, name="warm")
            for _ in range(3):
                nc.tensor.matmul(
                    warm_ps[:],
                    scratch[0:64, 0:C_OUT],
                    scratch[0:64, :],
                    start=True,
                    stop=True,
                )

            # Load the full padded images; early pieces small so matmuls
            # start (and never starve) early, then large contiguous pieces.
            # First piece of each image rides an otherwise-idle queue
            # (scalar / gpsimd) in parallel with the weight load on sync,
            # so the first real matmul can start ~1.5us earlier.
            row_splits = [0, 4, 12, 24, 40, 60, 84, 116, 152, 190, HP]
            for j in range(len(row_splits) - 1):
                a, b = row_splits[j], row_splits[j + 1]
                for img in range(IMGS_PER_CORE):
                    p0 = img * 64
                    eng = nc.sync
                    if j == 0:
                        eng = nc.scalar if img == 0 else nc.gpsimd
                    eng.dma_start(
                        out=xt[p0 : p0 + 64, 1 + a * wp : 1 + b * wp],
                        in_=x_d[img, :, a:b, :],
                    )

            for b in range(n_bands):
                y0 = b * r
                ost = [
                    o_pool.tile(
                        [C_OUT, flat], odt, tag=f"ost{img}", name=f"ost{img}_{b}"
                    )
                    for img in range(IMGS_PER_CORE)
                ]

                for c in range(n_chunk):
                    y = y0 + 2 * c
                    ps = [
                        p_pool.tile(
                            [C_OUT, chunk],
                            f32,
                            tag="ps",
                            bufs=8,
                            name=f"ps{i}_{b}_{c}",
                        )
                        for i in range(2)
                    ]
                    for t, (dr, dc) in enumerate(TAPS):
                        st = t == 0
                        sp = t == 8
                        base = (y + dr) * wp + dc
                        for img in range(IMGS_PER_CORE):
                            p0 = img * 64
                            rhs = xt[p0 : p0 + 64, base : base + 2 * wp].rearrange(
                                "p (a c) -> p a c", c=wp
                            )[:, :, 0:w]
                            nc.tensor.matmul(
                                ps[img][:],
                                w_sb[p0 : p0 + 64, t, :],
                                rhs,
                                start=st,
                                stop=sp,
                            )
                    for img in range(IMGS_PER_CORE):
                        # one eviction per engine per chunk: smoother DVE/ACT
                        # load and the last chunk's pair evicts in parallel
                        dst = ost[img][:, c * chunk : (c + 1) * chunk]
                        if img == 0:
                            nc.vector.tensor_scalar_add(dst, ps[img][:], bias_sb[:])
                        else:
                            nc.scalar.add(dst, ps[img][:], bias_sb[:])
                    # 8-row store pieces issued as soon as their chunks
                    # are evicted; the last band's final 8 rows go as two
                    # 4-row kicks so less transfer remains after the last
                    # matmul. (c, r_a, r_b): store rows [r_a, r_b) once
                    # chunk c is evicted.
                    if b == n_bands - 1:
                        pieces = {3: (0, 8), 7: (8, 16), 11: (16, 24),
                                  13: (24, 28), 15: (28, 32)}
                    else:
                        pieces = {3: (0, 8), 7: (8, 16), 11: (16, 24),
                                  15: (24, 32)}
                    if c in pieces:
                        r_a, r_b = pieces[c]
                        for img in range(IMGS_PER_CORE):
                            # gpsimd queue: a store kick that waits here
                            # must not head-of-line-block ACT evictions
                            # (scalar) or input loads (sync).
                            nc.gpsimd.dma_start(
                                out=out_d[img, :, y0 + r_a : y0 + r_b, :],
                                in_=ost[img][:, r_a * w : r_b * w],
                            )

    nc.compile()
    return nc


def prep_weight(weight: np.ndarray) -> np.ndarray:
    # [C_OUT, C_IN, 3, 3] -> [C_IN, 9, C_OUT]
    return np.ascontiguousarray(weight.transpose(1, 2, 3, 0).reshape(C_IN, 9, C_OUT))


def prep_x(x: np.ndarray) -> np.ndarray:
    # [N, C_IN, H, W] f32 -> padded [N, C_IN, HP, WP] f16 (zero halo rows
    # above/below, zero pad column on the right of every row).
    xp = np.zeros((N_IMG, C_IN, HP, WP), np.float16)
    xp[:, :, 1 : H + 1, :W] = x
    return xp


def run_conv(x, weight, bias, trace=False, out_bf16=True, evict_split=3):
    """x [16,64,224,224] f32. Returns (out [16,128,224,224] f32, results)."""
    from concourse.bass_utils import run_bass_kernel_spmd

    xp = prep_x(np.asarray(x, dtype=np.float32))
    w_t = prep_weight(np.asarray(weight, dtype=np.float32)).astype(np.float16)
    b_t = np.ascontiguousarray(np.asarray(bias, dtype=np.float32).reshape(C_OUT, 1))

    nc = build_conv_program(out_bf16=out_bf16, evict_split=evict_split)
    in_maps = [
        {
            "x": np.ascontiguousarray(xp[i * IMGS_PER_CORE : (i + 1) * IMGS_PER_CORE]),
            "w": w_t,
            "bias": b_t,
        }
        for i in range(N_CORES)
    ]
    res = run_bass_kernel_spmd(nc, in_maps, core_ids=list(range(N_CORES)), trace=trace)
    out = np.concatenate([r_["out"] for r_ in res.results], axis=0)
    if out.dtype != np.float32:
        out = out.astype(np.float32)
    return out, res


def kernel(**inputs) -> np.ndarray:
    out, _ = run_conv(inputs["x"], inputs["weight"], inputs["bias"])
    return out
